# revision 5
# baseline (speedup 1.0000x reference)
"""Trainium2 Bass kernel for the blocked-DCT corner-mask layer.

Math: for each 8x8 block B of the image, the reference computes
    coeffs = D^T B D        (2D DCT-II)
    out_c  = D (coeffs * mask_c) D^T   for 4 corner masks c
Each mask is an outer product of half-indicators, so with
    L = D[:, :4] @ D[:, :4].T   (symmetric projection),  H = I - L
the whole pipeline collapses to
    out_0 = L B L,  out_1 = L B H,  out_2 = H B L,  out_3 = H B H.

Per-8-row/8-col application over a full 512x512 image is multiplication by
the 128x128 block-diagonal BDL = blockdiag(L x 16) (symmetric) on either
side.  On-chip per [128, 512] tile X:
    [S^T|SH^T](c) = X[:, 128c:+128]^T @ [BDL | BDH]   (PE, N=256)
    [O0|O1](c) = S^T(c)^T-as-lhsT @ [BDL | BDH]       (PE, N=256)
    [O2|O3](c) = SH^T(c) likewise
Data always rides the lhsT (weights) slot so no explicit transposes are
needed; the constants stream as rhs.

I/O precision: bf16 on the wire (x, constants, outputs) with f32 PSUM
accumulation — halves HBM traffic vs f32 and stays ~0.3% rel err,
far inside the 2e-2 gate.  Host up/down-casts.

Sharding: data-parallel over batch, 4 batches (12 images) per core.
"""

import numpy as np
import ml_dtypes

FULL_B, DCH, H, W = 32, 3, 512, 512
N_CORES = 8
B_PER_CORE = FULL_B // N_CORES       # 4
IMGS = B_PER_CORE * DCH              # 12 images per core
P = 128

_BUILT = {}


def _consts() -> np.ndarray:
    """[128, 256] = [BDL | BDH] constants, computed in float64."""
    N = 8
    x = np.arange(N, dtype=np.float64)[:, None]
    u = np.arange(N, dtype=np.float64)[None, :]
    alpha = np.full(N, np.sqrt(2.0 / N))
    alpha[0] = np.sqrt(1.0 / N)
    D = alpha[None, :] * np.cos(np.pi * u * (2.0 * x + 1.0) / (2.0 * N))
    L = D[:, :4] @ D[:, :4].T
    Hm = np.eye(N) - L
    BDL = np.kron(np.eye(16), L)
    BDH = np.kron(np.eye(16), Hm)
    cat = np.concatenate([BDL, BDH], axis=1)
    return np.ascontiguousarray(cat.astype(ml_dtypes.bfloat16))


def _body(ctx, tc, o_ap, x_ap, c_ap, n_imgs):
    import concourse.mybir as mybir

    nc = tc.nc
    f32 = mybir.dt.float32
    bf16 = mybir.dt.bfloat16

    cpool = ctx.enter_context(tc.tile_pool(name="const", bufs=1))
    cst = cpool.tile([P, 256], bf16)
    nc.sync.dma_start(cst[:], c_ap[:, :])
    BDLH = cst[:, 0:256]  # packed [BDL | BDH] rhs, N=256

    sb = ctx.enter_context(tc.tile_pool(name="sb", bufs=1))
    ps = ctx.enter_context(tc.tile_pool(name="ps", bufs=1, space="PSUM"))

    def front(i):
        """input DMA + row-transform matmuls A = x^T @ [BDL|BDH] + copy.

        A-mm for chunk c: lhsT = X[:, 128c:128c+128] (contraction over
        image rows) -> out [128 = col-in-chunk, 256] = [S^T(c) | SH^T(c)]
        where S = BDL @ X, SH = BDH @ X.  No identity transposes needed.
        """
        img, t = divmod(i, 4)
        row = img * 512 + t * 128
        x_sb = sb.tile([P, 512], bf16, tag="x", bufs=10, name=f"x_{i}")
        nc.gpsimd.dma_start(x_sb[:], x_ap[row : row + 128, :])  # SWDGE ring

        a_ps = ps.tile([P, 1024], f32, tag="aps", bufs=2, name=f"aps_{i}")
        for c in range(4):
            nc.tensor.matmul(
                a_ps[:, 256 * c : 256 * (c + 1)],
                lhsT=x_sb[:, 128 * c : 128 * (c + 1)],
                rhs=BDLH,
                start=True,
                stop=True,
            )
        # split copy across both engines (different banks, concurrent)
        a_sb = sb.tile([P, 1024], bf16, tag="as", bufs=4, name=f"a_{i}")
        nc.vector.tensor_copy(a_sb[:, 0:512], a_ps[:, 0:512])  # DVE
        nc.scalar.copy(a_sb[:, 512:1024], a_ps[:, 512:1024])  # ACT
        return a_sb

    def back(i, a_sb):
        """output matmuls + de-interleaving copies + one merged output DMA.

        Out-mm chunk c: lhsT = S^T(c) -> [O0(c) | O1(c)] (natural
        orientation, contraction over columns = column transform);
        lhsT = SH^T(c) -> [O2(c) | O3(c)].
        """
        img, t = divmod(i, 4)
        a_v = a_sb[:].rearrange("p (c s l) -> p c s l", c=4, s=2, l=128)

        p01 = ps.tile([P, 1024], f32, tag="p01", bufs=1, name=f"p01_{i}")
        p23 = ps.tile([P, 1024], f32, tag="p23", bufs=1, name=f"p23_{i}")
        for c in range(4):
            nc.tensor.matmul(
                p01[:, 256 * c : 256 * (c + 1)],
                lhsT=a_v[:, c, 0, :],
                rhs=BDLH,
                start=True,
                stop=True,
            )  # [O0(c) | O1(c)]
            nc.tensor.matmul(
                p23[:, 256 * c : 256 * (c + 1)],
                lhsT=a_v[:, c, 1, :],
                rhs=BDLH,
                start=True,
                stop=True,
            )  # [O2(c) | O3(c)]

        # one SBUF tile holding all 4 corner rows [O0|O1|O2|O3]; 3D 512-col
        # strided copies stay on the DVE/ACT fast path (4D APs do not)
        o_sb = sb.tile([P, 2048], bf16, tag="o", bufs=5, name=f"o_{i}")
        p01_v = p01[:].rearrange("p (c s l) -> p c s l", c=4, s=2, l=128)
        p23_v = p23[:].rearrange("p (c s l) -> p c s l", c=4, s=2, l=128)
        for ci, (pv, half) in enumerate(
            [(p01_v, 0), (p01_v, 1), (p23_v, 0), (p23_v, 1)]
        ):
            dst = o_sb[:, 512 * ci : 512 * (ci + 1)].rearrange(
                "p (c l) -> p c l", c=4
            )
            if ci % 2 == 0:
                nc.vector.tensor_copy(dst, pv[:, :, half, :])  # DVE
            else:
                nc.scalar.copy(dst, pv[:, :, half, :])  # ACT

        # single DMA writes all 4 corner planes (sync HWDGE ring; the
        # sync sequencer is otherwise idle, keeping DMA-issue cost off
        # the copy engines)
        orow = img * 512 + t * 128
        dst = o_ap[:, orow : orow + 128, :].rearrange("q r c -> r q c")
        src = o_sb[:].rearrange("p (q l) -> p q l", q=4, l=512)
        nc.sync.dma_start(dst, src)

    # one-stage software skew: tile i's output stage is emitted after
    # tile i+1's front stage, keeping PE fed while PSUM banks drain
    ntiles = n_imgs * 4
    pending = None
    for i in range(ntiles):
        cch = front(i)
        if pending is not None:
            back(i - 1, pending)
        pending = cch
    back(ntiles - 1, pending)


def _build(n_imgs=IMGS):
    key = n_imgs
    if key in _BUILT:
        return _BUILT[key]
    from contextlib import ExitStack

    import concourse.bacc as bacc
    import concourse.mybir as mybir
    import concourse.tile as tile

    bf16 = mybir.dt.bfloat16
    nc = bacc.Bacc(
        "TRN2", target_bir_lowering=False, debug=False, num_devices=N_CORES
    )
    x_d = nc.dram_tensor("x", (n_imgs * 512, 512), bf16, kind="ExternalInput")
    c_d = nc.dram_tensor("cst", (P, 256), bf16, kind="ExternalInput")
    o_d = nc.dram_tensor("out", (4, n_imgs * 512, 512), bf16, kind="ExternalOutput")

    with tile.TileContext(nc) as tc:
        with ExitStack() as ctx:
            _body(ctx, tc, o_d.ap(), x_d.ap(), c_d.ap(), n_imgs)
    nc.compile()
    _BUILT[key] = nc
    return nc


def _run(x, trace=False):
    """x: (32, 3, 512, 512) float32. Returns (out, exec_time_ns)."""
    from concourse import bass_utils

    nc = _build(IMGS)
    consts = _consts()
    x16 = x.astype(ml_dtypes.bfloat16)
    in_maps = []
    for k in range(N_CORES):
        xs = x16[k * B_PER_CORE : (k + 1) * B_PER_CORE].reshape(IMGS * 512, 512)
        in_maps.append({"x": np.ascontiguousarray(xs), "cst": consts})
    res = bass_utils.run_bass_kernel_spmd(
        nc, in_maps, core_ids=list(range(N_CORES)), trace=trace
    )
    outs = []
    for k in range(N_CORES):
        o = res.results[k]["out"]
        o = np.asarray(o).astype(np.float32).reshape(4, B_PER_CORE, DCH, H, W)
        outs.append(o)
    full = np.concatenate(outs, axis=1)  # (4, 32, 3, 512, 512)
    return full, res.exec_time_ns


def kernel(**inputs) -> np.ndarray:
    x = np.ascontiguousarray(np.asarray(inputs["x"], dtype=np.float32))
    assert x.shape == (FULL_B, DCH, H, W), x.shape
    out, _ = _run(x, trace=False)
    return out


# revision 6
# speedup vs baseline: 1.2885x; 1.2885x over previous
"""Trainium2 Bass kernel for the blocked-DCT corner-mask layer.

Math: for each 8x8 block B of the image, the reference computes
    coeffs = D^T B D        (2D DCT-II)
    out_c  = D (coeffs * mask_c) D^T   for 4 corner masks c
Each mask is an outer product of half-indicators, so with
    L = D[:, :4] @ D[:, :4].T   (symmetric projection),  H = I - L
the whole pipeline collapses to
    out_0 = L B L,  out_1 = L B H,  out_2 = H B L,  out_3 = H B H.

Per-8-row/8-col application over a full 512x512 image is multiplication by
the 128x128 block-diagonal BDL = blockdiag(L x 16) (symmetric) on either
side.  On-chip per [128, 512] tile X:
    [S^T|SH^T](c) = X[:, 128c:+128]^T @ [BDL | BDH]   (PE, N=256)
    [O0|O1](c) = S^T(c)^T-as-lhsT @ [BDL | BDH]       (PE, N=256)
    [O2|O3](c) = SH^T(c) likewise
Data always rides the lhsT (weights) slot so no explicit transposes are
needed; the constants stream as rhs.

I/O precision: bf16 on the wire (x, constants, outputs) with f32 PSUM
accumulation — halves HBM traffic vs f32 and stays ~0.3% rel err,
far inside the 2e-2 gate.  Host up/down-casts.

Sharding: data-parallel over batch, 4 batches (12 images) per core.
"""

import numpy as np
import ml_dtypes

FULL_B, DCH, H, W = 32, 3, 512, 512
N_CORES = 8
B_PER_CORE = FULL_B // N_CORES       # 4
IMGS = B_PER_CORE * DCH              # 12 images per core
P = 128

_BUILT = {}


def _consts() -> np.ndarray:
    """[128, 256] = [BDL | BDH] constants, computed in float64."""
    N = 8
    x = np.arange(N, dtype=np.float64)[:, None]
    u = np.arange(N, dtype=np.float64)[None, :]
    alpha = np.full(N, np.sqrt(2.0 / N))
    alpha[0] = np.sqrt(1.0 / N)
    D = alpha[None, :] * np.cos(np.pi * u * (2.0 * x + 1.0) / (2.0 * N))
    L = D[:, :4] @ D[:, :4].T
    Hm = np.eye(N) - L
    BDL = np.kron(np.eye(16), L)
    BDH = np.kron(np.eye(16), Hm)
    cat = np.concatenate([BDL, BDH], axis=1)
    return np.ascontiguousarray(cat.astype(ml_dtypes.bfloat16))


def _body(ctx, tc, o_ap, x_ap, c_ap, n_imgs):
    import concourse.mybir as mybir

    nc = tc.nc
    f32 = mybir.dt.float32
    bf16 = mybir.dt.bfloat16

    cpool = ctx.enter_context(tc.tile_pool(name="const", bufs=1))
    cst = cpool.tile([P, 256], bf16)
    nc.sync.dma_start(cst[:], c_ap[:, :])
    BDLH = cst[:, 0:256]  # packed [BDL | BDH] rhs, N=256

    sb = ctx.enter_context(tc.tile_pool(name="sb", bufs=1))
    ps = ctx.enter_context(tc.tile_pool(name="ps", bufs=1, space="PSUM"))

    # half-tile pipeline: every PSUM tile is [128, 512] f32 = exactly one
    # bank, so aps(3) + p01h(2) + p23h(2) = 7 banks gives full double
    # buffering — back-MMs never wait on the previous iteration's copies.
    x_sbs, a_sbs, o_sbs = {}, {}, {}

    def front_half(j):
        """input DMA (once per tile) + 2 row-transform MMs + 1 copy.

        MM for chunk c: lhsT = X[:, 128c:128c+128] (contraction over
        image rows) -> out [128 = col-in-chunk, 256] = [S^T(c) | SH^T(c)]
        where S = BDL @ X, SH = BDH @ X.  No identity transposes needed.
        """
        i, h = divmod(j, 2)
        img, t = divmod(i, 4)
        if h == 0:
            row = img * 512 + t * 128
            x_sb = sb.tile([P, 512], bf16, tag="x", bufs=8, name=f"x_{i}")
            nc.gpsimd.dma_start(x_sb[:], x_ap[row : row + 128, :])  # SWDGE
            x_sbs[i] = x_sb
            a_sbs[i] = sb.tile([P, 1024], bf16, tag="as", bufs=4, name=f"a_{i}")
        x_sb, a_sb = x_sbs[i], a_sbs[i]

        aps = ps.tile([P, 512], f32, tag="aps", bufs=3, name=f"aps_{j}")
        for c2 in range(2):
            c = 2 * h + c2
            nc.tensor.matmul(
                aps[:, 256 * c2 : 256 * (c2 + 1)],
                lhsT=x_sb[:, 128 * c : 128 * (c + 1)],
                rhs=BDLH,
                start=True,
                stop=True,
            )
        # whole-half copy on one engine (alternating by h) so DVE and ACT
        # never read the same PSUM bank concurrently
        eng = nc.vector.tensor_copy if h == 0 else nc.scalar.copy
        eng(a_sb[:, 512 * h : 512 * (h + 1)], aps[:])

    def back_half(j):
        """4 output MMs + de-interleaving copies + output DMA (per tile).

        Out-mm chunk c: lhsT = S^T(c) -> [O0(c) | O1(c)] (natural
        orientation, contraction over columns = column transform);
        lhsT = SH^T(c) -> [O2(c) | O3(c)].
        """
        i, h = divmod(j, 2)
        img, t = divmod(i, 4)
        if h == 0:
            o_sbs[i] = sb.tile([P, 2048], bf16, tag="o", bufs=4, name=f"o_{i}")
        o_sb = o_sbs[i]
        a_v = a_sbs[i][:].rearrange("p (c s l) -> p c s l", c=4, s=2, l=128)

        p01 = ps.tile([P, 512], f32, tag="p01", bufs=2, name=f"p01_{j}")
        p23 = ps.tile([P, 512], f32, tag="p23", bufs=2, name=f"p23_{j}")
        for c2 in range(2):
            c = 2 * h + c2
            nc.tensor.matmul(
                p01[:, 256 * c2 : 256 * (c2 + 1)],
                lhsT=a_v[:, c, 0, :],
                rhs=BDLH,
                start=True,
                stop=True,
            )  # [O0(c) | O1(c)]
            nc.tensor.matmul(
                p23[:, 256 * c2 : 256 * (c2 + 1)],
                lhsT=a_v[:, c, 1, :],
                rhs=BDLH,
                start=True,
                stop=True,
            )  # [O2(c) | O3(c)]

        # o_sb column layout: 512*q + 256*h + 128*c2 + l  (q = corner)
        o_v = o_sb[:].rearrange("p (q x c l) -> p q x c l", q=4, x=2, c=2, l=128)
        p01_v = p01[:].rearrange("p (c s l) -> p s c l", c=2, s=2, l=128)
        p23_v = p23[:].rearrange("p (c s l) -> p s c l", c=2, s=2, l=128)
        nc.vector.tensor_copy(o_v[:, 0:2, h, :, :], p01_v)  # DVE: O0,O1
        nc.scalar.copy(o_v[:, 2:4, h, :, :], p23_v)  # ACT: O2,O3

        if h == 1:
            # single DMA writes all 4 corner planes (sync HWDGE ring;
            # the sync sequencer is otherwise idle, keeping DMA-issue
            # cost off the copy engines)
            orow = img * 512 + t * 128
            dst = o_ap[:, orow : orow + 128, :].rearrange("q r c -> r q c")
            src = o_sb[:].rearrange("p (q l) -> p q l", q=4, l=512)
            nc.sync.dma_start(dst, src)

    # two-half software skew keeps PE fed while PSUM banks drain
    nhalves = n_imgs * 4 * 2
    for j in range(nhalves):
        front_half(j)
        if j >= 2:
            back_half(j - 2)
    back_half(nhalves - 2)
    back_half(nhalves - 1)


def _build(n_imgs=IMGS):
    key = n_imgs
    if key in _BUILT:
        return _BUILT[key]
    from contextlib import ExitStack

    import concourse.bacc as bacc
    import concourse.mybir as mybir
    import concourse.tile as tile

    bf16 = mybir.dt.bfloat16
    nc = bacc.Bacc(
        "TRN2", target_bir_lowering=False, debug=False, num_devices=N_CORES
    )
    x_d = nc.dram_tensor("x", (n_imgs * 512, 512), bf16, kind="ExternalInput")
    c_d = nc.dram_tensor("cst", (P, 256), bf16, kind="ExternalInput")
    o_d = nc.dram_tensor("out", (4, n_imgs * 512, 512), bf16, kind="ExternalOutput")

    with tile.TileContext(nc) as tc:
        with ExitStack() as ctx:
            _body(ctx, tc, o_d.ap(), x_d.ap(), c_d.ap(), n_imgs)
    nc.compile()
    _BUILT[key] = nc
    return nc


def _run(x, trace=False):
    """x: (32, 3, 512, 512) float32. Returns (out, exec_time_ns)."""
    from concourse import bass_utils

    nc = _build(IMGS)
    consts = _consts()
    x16 = x.astype(ml_dtypes.bfloat16)
    in_maps = []
    for k in range(N_CORES):
        xs = x16[k * B_PER_CORE : (k + 1) * B_PER_CORE].reshape(IMGS * 512, 512)
        in_maps.append({"x": np.ascontiguousarray(xs), "cst": consts})
    res = bass_utils.run_bass_kernel_spmd(
        nc, in_maps, core_ids=list(range(N_CORES)), trace=trace
    )
    outs = []
    for k in range(N_CORES):
        o = res.results[k]["out"]
        o = np.asarray(o).astype(np.float32).reshape(4, B_PER_CORE, DCH, H, W)
        outs.append(o)
    full = np.concatenate(outs, axis=1)  # (4, 32, 3, 512, 512)
    return full, res.exec_time_ns


def kernel(**inputs) -> np.ndarray:
    x = np.ascontiguousarray(np.asarray(inputs["x"], dtype=np.float32))
    assert x.shape == (FULL_B, DCH, H, W), x.shape
    out, _ = _run(x, trace=False)
    return out


# revision 7
# speedup vs baseline: 1.4237x; 1.1050x over previous
"""Trainium2 Bass kernel for the blocked-DCT corner-mask layer.

Math: for each 8x8 block B of the image, the reference computes
    coeffs = D^T B D        (2D DCT-II)
    out_c  = D (coeffs * mask_c) D^T   for 4 corner masks c
Each mask is an outer product of half-indicators, so with
    L = D[:, :4] @ D[:, :4].T   (symmetric projection),  H = I - L
the whole pipeline collapses to
    out_0 = L B L,  out_1 = L B H,  out_2 = H B L,  out_3 = H B H.

Per-8-row/8-col application over a full 512x512 image is multiplication by
the 128x128 block-diagonal BDL = blockdiag(L x 16) (symmetric) on either
side.  On-chip per [128, 512] tile X:
    [S^T|SH^T](c) = X[:, 128c:+128]^T @ [BDL | BDH]   (PE, N=256)
    [O0|O1](c) = S^T(c)^T-as-lhsT @ [BDL | BDH]       (PE, N=256)
    [O2|O3](c) = SH^T(c) likewise
Data always rides the lhsT (weights) slot so no explicit transposes are
needed; the constants stream as rhs.

I/O precision: bf16 on the wire (x, constants, outputs) with f32 PSUM
accumulation — halves HBM traffic vs f32 and stays ~0.3% rel err,
far inside the 2e-2 gate.  Host up/down-casts.

Sharding: data-parallel over batch, 4 batches (12 images) per core.
"""

import numpy as np
import ml_dtypes

FULL_B, DCH, H, W = 32, 3, 512, 512
N_CORES = 8
B_PER_CORE = FULL_B // N_CORES       # 4
IMGS = B_PER_CORE * DCH              # 12 images per core
P = 128

_BUILT = {}


def _consts() -> np.ndarray:
    """[128, 256] = [BDL | BDH] constants, computed in float64."""
    N = 8
    x = np.arange(N, dtype=np.float64)[:, None]
    u = np.arange(N, dtype=np.float64)[None, :]
    alpha = np.full(N, np.sqrt(2.0 / N))
    alpha[0] = np.sqrt(1.0 / N)
    D = alpha[None, :] * np.cos(np.pi * u * (2.0 * x + 1.0) / (2.0 * N))
    L = D[:, :4] @ D[:, :4].T
    Hm = np.eye(N) - L
    BDL = np.kron(np.eye(16), L)
    BDH = np.kron(np.eye(16), Hm)
    cat = np.concatenate([BDL, BDH], axis=1)
    return np.ascontiguousarray(cat.astype(ml_dtypes.bfloat16))


def _body(ctx, tc, o_ap, x_ap, c_ap, n_imgs):
    import concourse.mybir as mybir

    nc = tc.nc
    f32 = mybir.dt.float32
    bf16 = mybir.dt.bfloat16

    cpool = ctx.enter_context(tc.tile_pool(name="const", bufs=1))
    cst = cpool.tile([P, 256], bf16)
    nc.sync.dma_start(cst[:], c_ap[:, :])
    BDLH = cst[:, 0:256]  # packed [BDL | BDH] rhs, N=256

    sb = ctx.enter_context(tc.tile_pool(name="sb", bufs=1))
    ps = ctx.enter_context(tc.tile_pool(name="ps", bufs=1, space="PSUM"))

    # half-tile pipeline: every PSUM tile is [128, 512] f32 = exactly one
    # bank, so aps(3) + p01h(2) + p23h(2) = 7 banks gives full double
    # buffering — back-MMs never wait on the previous iteration's copies.
    x_sbs, a_sbs, o_sbs = {}, {}, {}

    def front_half(j):
        """input DMA (once per tile) + 2 row-transform MMs + 1 copy.

        MM for chunk c: lhsT = X[:, 128c:128c+128] (contraction over
        image rows) -> out [128 = col-in-chunk, 256] = [S^T(c) | SH^T(c)]
        where S = BDL @ X, SH = BDH @ X.  No identity transposes needed.
        """
        i, h = divmod(j, 2)
        img, t = divmod(i, 4)
        if h == 0:
            row = img * 512 + t * 128
            x_sb = sb.tile([P, 512], bf16, tag="x", bufs=10, name=f"x_{i}")
            nc.gpsimd.dma_start(x_sb[:], x_ap[row : row + 128, :])  # SWDGE
            x_sbs[i] = x_sb
            a_sbs[i] = sb.tile([P, 1024], bf16, tag="as", bufs=6, name=f"a_{i}")
        x_sb, a_sb = x_sbs[i], a_sbs[i]

        aps = ps.tile([P, 512], f32, tag="aps", bufs=4, name=f"aps_{j}")
        for c2 in range(2):
            c = 2 * h + c2
            nc.tensor.matmul(
                aps[:, 256 * c2 : 256 * (c2 + 1)],
                lhsT=x_sb[:, 128 * c : 128 * (c + 1)],
                rhs=BDLH,
                start=True,
                stop=True,
            )
        # whole-half copy on one engine (alternating by h) so DVE and ACT
        # never read the same PSUM bank concurrently
        eng = nc.vector.tensor_copy if h == 0 else nc.scalar.copy
        eng(a_sb[:, 512 * h : 512 * (h + 1)], aps[:])

    def back_half(j):
        """4 output MMs + de-interleaving copies + output DMA (per tile).

        Out-mm chunk c: lhsT = S^T(c) -> [O0(c) | O1(c)] (natural
        orientation, contraction over columns = column transform);
        lhsT = SH^T(c) -> [O2(c) | O3(c)].
        """
        i, h = divmod(j, 2)
        img, t = divmod(i, 4)
        if h == 0:
            o_sbs[i] = sb.tile([P, 2048], bf16, tag="o", bufs=6, name=f"o_{i}")
        o_sb = o_sbs[i]
        a_v = a_sbs[i][:].rearrange("p (c s l) -> p c s l", c=4, s=2, l=128)

        p01 = ps.tile([P, 512], f32, tag="p01", bufs=2, name=f"p01_{j}")
        p23 = ps.tile([P, 512], f32, tag="p23", bufs=2, name=f"p23_{j}")
        for c2 in range(2):
            c = 2 * h + c2
            nc.tensor.matmul(
                p01[:, 256 * c2 : 256 * (c2 + 1)],
                lhsT=a_v[:, c, 0, :],
                rhs=BDLH,
                start=True,
                stop=True,
            )  # [O0(c) | O1(c)]
            nc.tensor.matmul(
                p23[:, 256 * c2 : 256 * (c2 + 1)],
                lhsT=a_v[:, c, 1, :],
                rhs=BDLH,
                start=True,
                stop=True,
            )  # [O2(c) | O3(c)]

        # o_sb column layout: 512*q + 256*h + 128*c2 + l  (q = corner)
        o_v = o_sb[:].rearrange("p (q x c l) -> p q x c l", q=4, x=2, c=2, l=128)
        p01_v = p01[:].rearrange("p (c s l) -> p s c l", c=2, s=2, l=128)
        p23_v = p23[:].rearrange("p (c s l) -> p s c l", c=2, s=2, l=128)
        nc.vector.tensor_copy(o_v[:, 0:2, h, :, :], p01_v)  # DVE: O0,O1
        nc.scalar.copy(o_v[:, 2:4, h, :, :], p23_v)  # ACT: O2,O3

        if h == 1:
            # single DMA writes all 4 corner planes (sync HWDGE ring;
            # the sync sequencer is otherwise idle, keeping DMA-issue
            # cost off the copy engines)
            orow = img * 512 + t * 128
            dst = o_ap[:, orow : orow + 128, :].rearrange("q r c -> r q c")
            src = o_sb[:].rearrange("p (q l) -> p q l", q=4, l=512)
            nc.sync.dma_start(dst, src)

    # three-half software skew keeps PE fed while PSUM banks drain
    nhalves = n_imgs * 4 * 2
    SKEW = 3
    for j in range(nhalves):
        front_half(j)
        if j >= SKEW:
            back_half(j - SKEW)
    for j in range(nhalves - SKEW, nhalves):
        back_half(j)


def _build(n_imgs=IMGS):
    key = n_imgs
    if key in _BUILT:
        return _BUILT[key]
    from contextlib import ExitStack

    import concourse.bacc as bacc
    import concourse.mybir as mybir
    import concourse.tile as tile

    bf16 = mybir.dt.bfloat16
    nc = bacc.Bacc(
        "TRN2", target_bir_lowering=False, debug=False, num_devices=N_CORES
    )
    x_d = nc.dram_tensor("x", (n_imgs * 512, 512), bf16, kind="ExternalInput")
    c_d = nc.dram_tensor("cst", (P, 256), bf16, kind="ExternalInput")
    o_d = nc.dram_tensor("out", (4, n_imgs * 512, 512), bf16, kind="ExternalOutput")

    with tile.TileContext(nc) as tc:
        with ExitStack() as ctx:
            _body(ctx, tc, o_d.ap(), x_d.ap(), c_d.ap(), n_imgs)
    nc.compile()
    _BUILT[key] = nc
    return nc


def _run(x, trace=False):
    """x: (32, 3, 512, 512) float32. Returns (out, exec_time_ns)."""
    from concourse import bass_utils

    nc = _build(IMGS)
    consts = _consts()
    x16 = x.astype(ml_dtypes.bfloat16)
    in_maps = []
    for k in range(N_CORES):
        xs = x16[k * B_PER_CORE : (k + 1) * B_PER_CORE].reshape(IMGS * 512, 512)
        in_maps.append({"x": np.ascontiguousarray(xs), "cst": consts})
    res = bass_utils.run_bass_kernel_spmd(
        nc, in_maps, core_ids=list(range(N_CORES)), trace=trace
    )
    outs = []
    for k in range(N_CORES):
        o = res.results[k]["out"]
        o = np.asarray(o).astype(np.float32).reshape(4, B_PER_CORE, DCH, H, W)
        outs.append(o)
    full = np.concatenate(outs, axis=1)  # (4, 32, 3, 512, 512)
    return full, res.exec_time_ns


def kernel(**inputs) -> np.ndarray:
    x = np.ascontiguousarray(np.asarray(inputs["x"], dtype=np.float32))
    assert x.shape == (FULL_B, DCH, H, W), x.shape
    out, _ = _run(x, trace=False)
    return out
